# revision 1
# baseline (speedup 1.0000x reference)
"""Trainium2 Bass kernel for nn_Loss_2 (weighted BCE + index-gathered CE mean).

Data-parallel over 8 NeuronCores: each core processes 8 of the 64 batches,
computes per-partition partial sums on-chip, host sums 8x[128,1] partials and
divides by B*S.

Per-core program (tokens laid out [NT, 128, Tp] contiguous):
  LnC  = Ln(comb)                          (ScalarE, bf16)
  idxg = y_comb + (1-ys)*64                (pushes ys==0 tokens out of [0,20))
  D    = idxg_bcast - iota_class           (DVE, bf16; ==0 exactly at gathered class)
  cce_p = sum((D==0) * LnC)                (DVE scalar_tensor_tensor + accum_out)
  b1_p  = sum((ys*-W1) * Ln(ps))           (DVE scalar_tensor_tensor + accum_out)
  b0_p  = sum(((1-ys)*-W0) * Ln(1-ps))     (DVE scalar_tensor_tensor + accum_out)
  acc  += b1_p + b0_p - cce_p
"""

import sys

if '/opt/trn_rl_repo' not in sys.path:
    sys.path.insert(0, '/opt/trn_rl_repo')

import numpy as np

import concourse.bass as bass
import concourse.bacc as bacc
import concourse.tile as tile
import concourse.mybir as mybir
from concourse.bass_utils import run_bass_kernel_spmd

F32 = mybir.dt.float32
BF16 = mybir.dt.bfloat16
I32 = mybir.dt.int32
I16 = mybir.dt.int16

B, S, C = 64, 16384, 20
W0, W1 = 0.51, 19.05
BIG = 64.0
P = 128
N_CORES = 8
Tp = 256                       # tokens per partition per tile
NT = (B // N_CORES) * S // (P * Tp)  # 4 tiles per core


def _build(NT, Tp, comb_bufs=2):
    FREE = Tp * C
    nc = bacc.Bacc("TRN2", target_bir_lowering=False, debug=False)

    comb_d = nc.dram_tensor("comb", [NT, P, FREE], F32, kind="ExternalInput").ap()
    idxg_d = nc.dram_tensor("idxg", [NT, P, Tp], F32, kind="ExternalInput").ap()
    ps_d = nc.dram_tensor("ps", [NT, P, Tp], F32, kind="ExternalInput").ap()
    ys_d = nc.dram_tensor("ys", [NT, P, Tp], F32, kind="ExternalInput").ap()
    out_d = nc.dram_tensor("out", [P, 1], F32, kind="ExternalOutput").ap()

    with tile.TileContext(nc) as tc:
        with (
            tc.tile_pool(name="const", bufs=1) as const_pool,
            tc.tile_pool(name="comb", bufs=comb_bufs) as comb_pool,
            tc.tile_pool(name="big", bufs=2) as big_pool,
            tc.tile_pool(name="small", bufs=3) as small_pool,
        ):
            iota_t = const_pool.tile([P, FREE], I16)
            nc.gpsimd.iota(iota_t[:], pattern=[[0, Tp], [1, C]], base=0,
                           channel_multiplier=0)
            iota_v = iota_t[:].rearrange("p (t c) -> p t c", c=C)

            partsA = const_pool.tile([P, 2 * NT], F32)
            partsB = const_pool.tile([P, NT], F32)

            for i in range(NT):
                comb_t = comb_pool.tile([P, FREE], F32, tag="comb")
                nc.sync.dma_start(comb_t[:], comb_d[i])
                idxg = small_pool.tile([P, Tp], F32, tag="idxg")
                nc.sync.dma_start(idxg[:], idxg_d[i])
                ps_t = small_pool.tile([P, Tp], F32, tag="ps")
                nc.sync.dma_start(ps_t[:], ps_d[i])
                ys_t = small_pool.tile([P, Tp], F32, tag="ys")
                nc.sync.dma_start(ys_t[:], ys_d[i])

                lnc = big_pool.tile([P, FREE], BF16, tag="lnc")
                nc.scalar.activation(lnc[:], comb_t[:], mybir.ActivationFunctionType.Ln)

                idxg_b = idxg[:].rearrange("p (t o) -> p t o", o=1)

                mask = big_pool.tile([P, FREE], BF16, tag="mask")
                mask_v = mask[:].rearrange("p (t c) -> p t c", c=C)
                b_iota, b_idxg = bass.broadcast_tensor_aps(iota_v, idxg_b)
                nc.vector.tensor_tensor(mask_v, b_iota, b_idxg,
                                        mybir.AluOpType.is_equal)

                if False:
                    # DVE-only path: fused mult+sum on VectorE
                    nc.vector.scalar_tensor_tensor(
                        mask[:], mask[:], 1.0, lnc[:],
                        op0=mybir.AluOpType.mult, op1=mybir.AluOpType.mult,
                        accum_out=partsB[:, i:i + 1],
                    )
                else:
                    # split path: 2x bf16 multiply on DVE, sum on ScalarE
                    prod = big_pool.tile([P, FREE], BF16, tag="prod")
                    nc.vector.tensor_tensor(prod[:], mask[:], lnc[:],
                                            mybir.AluOpType.mult)
                    nc.scalar.activation(prod[:], prod[:],
                                         mybir.ActivationFunctionType.Copy,
                                         accum_out=partsB[:, i:i + 1])

                lps = small_pool.tile([P, Tp], F32, tag="lps")
                nc.scalar.activation(lps[:], ps_t[:], mybir.ActivationFunctionType.Ln)
                l1m = small_pool.tile([P, Tp], F32, tag="l1m")
                nc.scalar.activation(l1m[:], ps_t[:], mybir.ActivationFunctionType.Ln,
                                     bias=1.0, scale=-1.0)

                nc.vector.scalar_tensor_tensor(
                    lps[:], ys_t[:], -W1, lps[:],
                    op0=mybir.AluOpType.mult, op1=mybir.AluOpType.mult,
                    accum_out=partsA[:, 2 * i:2 * i + 1],
                )

                ys1m = small_pool.tile([P, Tp], F32, tag="ys1m")
                nc.vector.tensor_scalar(ys1m[:], ys_t[:], -1.0, 1.0,
                                        mybir.AluOpType.mult, mybir.AluOpType.add)
                nc.vector.scalar_tensor_tensor(
                    l1m[:], ys1m[:], -W0, l1m[:],
                    op0=mybir.AluOpType.mult, op1=mybir.AluOpType.mult,
                    accum_out=partsA[:, 2 * i + 1:2 * i + 2],
                )

            rA = const_pool.tile([P, 1], F32)
            nc.vector.tensor_reduce(rA[:], partsA[:], axis=mybir.AxisListType.X,
                                    op=mybir.AluOpType.add)
            rB = const_pool.tile([P, 1], F32)
            nc.vector.tensor_reduce(rB[:], partsB[:], axis=mybir.AxisListType.X,
                                    op=mybir.AluOpType.add)
            total = const_pool.tile([P, 1], F32)
            nc.vector.tensor_tensor(total[:], rA[:], rB[:], mybir.AluOpType.subtract)

            nc.sync.dma_start(out_d[:], total[:])

    nc.compile()
    return nc


_NC_CACHE = {}
IOTAC = np.ascontiguousarray(
    np.broadcast_to(np.tile(np.arange(C, dtype=np.int16), Tp), (P, Tp * C)))


def make_in_maps(y_pred_stroke, y_pred_comb, y_stroke, y_comb):
    y_pred_stroke = np.asarray(y_pred_stroke, dtype=np.float32)
    y_pred_comb = np.asarray(y_pred_comb, dtype=np.float32)
    y_stroke = np.asarray(y_stroke, dtype=np.float32)
    y_comb = np.asarray(y_comb)
    FREE = Tp * C
    Bc = B // N_CORES
    in_maps = []
    for c in range(N_CORES):
        sl = slice(c * Bc, (c + 1) * Bc)
        in_maps.append({
            "comb": np.ascontiguousarray(y_pred_comb[sl]).reshape(NT, P, FREE),
            "idxg": (np.ascontiguousarray(y_comb[sl]).astype(np.float32)
                     + (1.0 - np.ascontiguousarray(y_stroke[sl])[..., 0]) * BIG
                     ).reshape(NT, P, Tp),
            "ps": np.ascontiguousarray(y_pred_stroke[sl]).reshape(NT, P, Tp),
            "ys": np.ascontiguousarray(y_stroke[sl]).reshape(NT, P, Tp),
        })
    return in_maps


def kernel(y_pred_stroke, y_pred_comb, y_stroke, y_comb):
    key = (NT, Tp)
    if key not in _NC_CACHE:
        _NC_CACHE[key] = _build(NT, Tp)
    nc = _NC_CACHE[key]
    in_maps = make_in_maps(y_pred_stroke, y_pred_comb, y_stroke, y_comb)
    res = run_bass_kernel_spmd(nc, in_maps, list(range(N_CORES)))
    total = 0.0
    for r in res.results:
        total += r["out"].astype(np.float64).sum()
    return np.asarray([total / (B * S)], dtype=np.float32)



# revision 2
# speedup vs baseline: 1.2160x; 1.2160x over previous
"""Trainium2 Bass kernel for nn_Loss_2 (weighted BCE + index-gathered CE mean).

Data-parallel over 8 NeuronCores: each core processes 8 of the 64 batches
(131072 tokens = [P=128, T=1024]), computes per-partition partial sums
on-chip; host sums 8x[128,1] partials and divides by B*S.

Layout/precision strategy (memory-bound problem):
  comb  : fp8-e5m2, class-major [G, P, Cg, T] (G=4 groups of Cg=5 classes)
          -> 1 byte/elem HBM traffic; ScalarE reads fp8 directly for Ln.
  sides : bf16 [P, 2, T] = (idxg, ps) where idxg = y_comb + 64*(1-ys)
          pushes ys==0 tokens out of [0,20) so they match no class; ys is
          recovered on-device as (idxg < 32).

Per-core program:
  lps   = Ln(ps), l1m = Ln(1 - ps)                  (ScalarE)
  ys    = (idxg < 32); ys1m = (idxg >= 32)          (DVE tensor_scalar)
  pA0   = sum((ys * -W1) * lps)                     (DVE stt + accum_out)
  pA1   = sum((ys1m * -W0) * l1m)                   (DVE stt + accum_out)
  per class group g (pipelined with DMA):
    lnc_g = Ln(comb_f8[g])                          (ScalarE, one call)
    per class c: pB[cls] = sum((idxg == cls) * lnc_g[:, c, :])
                                                    (DVE stt + accum_out, 2x bf16)
  total = (pA0 + pA1) - sum_c pB[c]
"""

import sys

if '/opt/trn_rl_repo' not in sys.path:
    sys.path.insert(0, '/opt/trn_rl_repo')

import numpy as np
import ml_dtypes

import concourse.bass as bass
import concourse.bacc as bacc
import concourse.tile as tile
import concourse.mybir as mybir
from concourse.bass_utils import run_bass_kernel_spmd

F32 = mybir.dt.float32
BF16 = mybir.dt.bfloat16
F8E5 = mybir.dt.float8e5

B, S, C = 64, 16384, 20
W0, W1 = 0.51, 19.05
BIG = 64.0
P = 128
N_CORES = 8
T = (B // N_CORES) * S // P        # 1024 tokens per partition per core
G = 4                              # class groups
Cg = C // G                        # classes per group

NT = G                             # kept for test.py cache-key compat
Tp = T


def _build(G_, T_):
    nc = bacc.Bacc("TRN2", target_bir_lowering=False, debug=False)

    comb_d = nc.dram_tensor("comb", [G_, P, Cg * T_], F8E5, kind="ExternalInput").ap()
    sides_d = nc.dram_tensor("sides", [P, 2 * T_], BF16, kind="ExternalInput").ap()
    out_d = nc.dram_tensor("out", [P, 1], F32, kind="ExternalOutput").ap()

    eq = mybir.AluOpType.is_equal
    mul = mybir.AluOpType.mult
    Ln = mybir.ActivationFunctionType.Ln

    with tile.TileContext(nc) as tc:
        with (
            tc.tile_pool(name="const", bufs=1) as const_pool,
            tc.tile_pool(name="comb", bufs=2) as comb_pool,
            tc.tile_pool(name="lnc", bufs=2) as lnc_pool,
            tc.tile_pool(name="small", bufs=1) as small_pool,
            tc.tile_pool(name="scr", bufs=3) as scr_pool,
        ):
            parts = const_pool.tile([P, C + 2], F32)

            sides_t = small_pool.tile([P, 2 * T_], BF16, tag="sides")
            nc.sync.dma_start(sides_t[:], sides_d[:])
            idxg = sides_t[:, 0:T_]
            ps = sides_t[:, T_:2 * T_]

            # BCE side: ys from idxg sentinel, logs on ScalarE, fused
            # weighted accumulation on DVE.
            ys = small_pool.tile([P, T_], BF16, tag="ys")
            nc.vector.tensor_scalar(ys[:], idxg, 32.0, None, mybir.AluOpType.is_lt)
            ys1m = small_pool.tile([P, T_], BF16, tag="ys1m")
            nc.vector.tensor_scalar(ys1m[:], idxg, 32.0, None, mybir.AluOpType.is_ge)

            lps = small_pool.tile([P, T_], BF16, tag="lps")
            nc.scalar.activation(lps[:], ps, Ln)
            l1m = small_pool.tile([P, T_], BF16, tag="l1m")
            nc.scalar.activation(l1m[:], ps, Ln, bias=1.0, scale=-1.0)

            sA = scr_pool.tile([P, T_], BF16, tag="scr")
            nc.vector.scalar_tensor_tensor(
                sA[:], ys[:], -W1, lps[:], op0=mul, op1=mul,
                accum_out=parts[:, C:C + 1])
            sB = scr_pool.tile([P, T_], BF16, tag="scr")
            nc.vector.scalar_tensor_tensor(
                sB[:], ys1m[:], -W0, l1m[:], op0=mul, op1=mul,
                accum_out=parts[:, C + 1:C + 2])

            # CCE: per class group, Ln on ScalarE then per-class fused
            # (idxg == cls) * lnc accumulation on DVE.
            for g in range(G_):
                comb_t = comb_pool.tile([P, Cg * T_], F8E5, tag="comb")
                nc.sync.dma_start(comb_t[:], comb_d[g])
                lnc = lnc_pool.tile([P, Cg * T_], BF16, tag="lnc")
                nc.scalar.activation(lnc[:], comb_t[:], Ln)
                for c in range(Cg):
                    cls = g * Cg + c
                    scr = scr_pool.tile([P, T_], BF16, tag="scr")
                    nc.vector.scalar_tensor_tensor(
                        scr[:], idxg, float(cls), lnc[:, c * T_:(c + 1) * T_],
                        op0=eq, op1=mul,
                        accum_out=parts[:, cls:cls + 1])

            rB = const_pool.tile([P, 1], F32)
            nc.vector.tensor_reduce(rB[:], parts[:, 0:C], axis=mybir.AxisListType.X,
                                    op=mybir.AluOpType.add)
            rA = const_pool.tile([P, 1], F32)
            nc.vector.tensor_reduce(rA[:], parts[:, C:C + 2], axis=mybir.AxisListType.X,
                                    op=mybir.AluOpType.add)
            total = const_pool.tile([P, 1], F32)
            nc.vector.tensor_tensor(total[:], rA[:], rB[:], mybir.AluOpType.subtract)

            nc.sync.dma_start(out_d[:], total[:])

    nc.compile()
    return nc


_NC_CACHE = {}


def make_in_maps(y_pred_stroke, y_pred_comb, y_stroke, y_comb):
    y_pred_stroke = np.asarray(y_pred_stroke, dtype=np.float32)
    y_pred_comb = np.asarray(y_pred_comb, dtype=np.float32)
    y_stroke = np.asarray(y_stroke, dtype=np.float32)
    y_comb = np.asarray(y_comb)
    Bc = B // N_CORES
    in_maps = []
    for core in range(N_CORES):
        sl = slice(core * Bc, (core + 1) * Bc)
        # class-major fp8: [8,16384,20] -> [P, T, C] -> [C-major groups]
        cm = np.ascontiguousarray(y_pred_comb[sl]).reshape(P, T, C)
        cm = np.ascontiguousarray(cm.transpose(0, 2, 1))          # [P, C, T]
        comb8 = cm.astype(ml_dtypes.float8_e5m2).reshape(P, G, Cg * T)
        comb8 = np.ascontiguousarray(comb8.transpose(1, 0, 2))    # [G, P, Cg*T]

        idxg = (np.ascontiguousarray(y_comb[sl]).astype(np.float32)
                + (1.0 - np.ascontiguousarray(y_stroke[sl])[..., 0]) * BIG
                ).reshape(P, T)
        ps = np.ascontiguousarray(y_pred_stroke[sl]).reshape(P, T)
        sides = np.concatenate([idxg, ps], axis=1).astype(ml_dtypes.bfloat16)
        in_maps.append({"comb": comb8, "sides": sides})
    return in_maps


def kernel(y_pred_stroke, y_pred_comb, y_stroke, y_comb):
    key = (NT, Tp)
    if key not in _NC_CACHE:
        _NC_CACHE[key] = _build(G, T)
    nc = _NC_CACHE[key]
    in_maps = make_in_maps(y_pred_stroke, y_pred_comb, y_stroke, y_comb)
    res = run_bass_kernel_spmd(nc, in_maps, list(range(N_CORES)))
    total = 0.0
    for r in res.results:
        total += r["out"].astype(np.float64).sum()
    return np.asarray([total / (B * S)], dtype=np.float32)


# revision 6
# speedup vs baseline: 1.2291x; 1.0108x over previous
"""Trainium2 Bass kernel for nn_Loss_2 (weighted BCE + index-gathered CE mean).

Data-parallel over 8 NeuronCores: each core processes 8 of the 64 batches
(131072 tokens = [P=128, T=1024]), computes per-partition partial sums
on-chip; host sums 8x[128,1] partials and divides by B*S.

Strategy (few bulk instructions; per-instruction sync is ~250ns):
  comb is laid out block-token-major [nblk, C, B] per partition (B=4
  tokens per block) so that both operands of every bulk DVE op have a
  packed 2-byte last dim (2x DVE rate):
    nem     = (iota_class != idxg)        broadcast-3D TT, bf16
    masked1 = max(nem, comb)              -> comb[t,c] where class matches,
                                             1.0 elsewhere
    cce_sum = accum(Ln(masked1))          ScalarE, ln(1)=0 for non-matches
  ys==0 tokens have idxg pushed out of [0,20) (sentinel +64) so they match
  no class and contribute ln(1)=0.
  BCE uses the same select-by-max trick:
    m1 = max(1-ys, ps)   -> ps where ys==1 else 1 ; accum(Ln) * W1
    m0 = max(ys, 1-ps)   -> 1-ps where ys==0 else 1 ; accum(Ln) * W0
  total = -W1*a1 - W0*a0 - cce_sum
"""

import sys

if '/opt/trn_rl_repo' not in sys.path:
    sys.path.insert(0, '/opt/trn_rl_repo')

import numpy as np
import ml_dtypes

import concourse.bass as bass
import concourse.bacc as bacc
import concourse.tile as tile
import concourse.mybir as mybir
from concourse.bass_utils import run_bass_kernel_spmd

F32 = mybir.dt.float32
BF16 = mybir.dt.bfloat16

B, S, C = 64, 16384, 20
W0, W1 = 0.51, 19.05
BIG = 64.0
P = 128
N_CORES = 8
T = (B // N_CORES) * S // P        # 1024 tokens per partition per core
BLK = 4                            # tokens per block (packed last dim)
NH = 2                             # halves for DMA/compute pipelining
TH = T // NH                       # tokens per partition per half
NBLK = TH // BLK                   # blocks per half

NT = NH                            # kept for test.py cache-key compat
Tp = T


def _build(NH_, T_):
    nc = bacc.Bacc("TRN2", target_bir_lowering=False, debug=False)

    FREE_H = NBLK * C * BLK        # comb elements per partition per half
    comb_d = nc.dram_tensor("comb", [NH_, P, FREE_H], BF16, kind="ExternalInput").ap()
    sides_d = nc.dram_tensor("sides", [P, 2 * T_], BF16, kind="ExternalInput").ap()
    out_d = nc.dram_tensor("out", [P, 1], F32, kind="ExternalOutput").ap()

    ne = mybir.AluOpType.not_equal
    mx = mybir.AluOpType.max
    Ln = mybir.ActivationFunctionType.Ln

    with tile.TileContext(nc) as tc:
        with (
            tc.tile_pool(name="const", bufs=1) as const_pool,
            tc.tile_pool(name="comb", bufs=2) as comb_pool,
            tc.tile_pool(name="nem", bufs=2) as nem_pool,
            tc.tile_pool(name="msk", bufs=2) as msk_pool,
            tc.tile_pool(name="small", bufs=1) as small_pool,
        ):
            parts = const_pool.tile([P, NH_ + 2], F32)

            sides_t = small_pool.tile([P, 2 * T_], BF16, tag="sides")
            nc.sync.dma_start(sides_t[:], sides_d[:])
            idxg = sides_t[:, 0:T_]
            ps = sides_t[:, T_:2 * T_]

            # iota over classes, broadcast over blocks: value = class index
            iota_cb = const_pool.tile([P, C * BLK], BF16)
            nc.gpsimd.iota(iota_cb[:], pattern=[[1, C], [0, BLK]], base=0,
                           channel_multiplier=0,
                           allow_small_or_imprecise_dtypes=True)

            # BCE: select-by-max then Ln+accum (weights applied at combine)
            ys = small_pool.tile([P, T_], BF16, tag="ys")
            nc.vector.tensor_scalar(ys[:], idxg, 32.0, None, mybir.AluOpType.is_lt)
            ys1m = small_pool.tile([P, T_], BF16, tag="ys1m")
            nc.vector.tensor_scalar(ys1m[:], idxg, 32.0, None, mybir.AluOpType.is_ge)
            ps1m = small_pool.tile([P, T_], BF16, tag="ps1m")
            nc.vector.tensor_scalar(ps1m[:], ps, -1.0, 1.0,
                                    mybir.AluOpType.mult, mybir.AluOpType.add)

            m1 = small_pool.tile([P, T_], BF16, tag="m1")
            nc.vector.tensor_tensor(m1[:], ys1m[:], ps, mx)
            m0 = small_pool.tile([P, T_], BF16, tag="m0")
            nc.vector.tensor_tensor(m0[:], ys[:], ps1m[:], mx)

            j1 = small_pool.tile([P, T_], BF16, tag="j1")
            nc.scalar.activation(j1[:], m1[:], Ln, accum_out=parts[:, NH_:NH_ + 1])
            j0 = small_pool.tile([P, T_], BF16, tag="j0")
            nc.scalar.activation(j0[:], m0[:], Ln,
                                 accum_out=parts[:, NH_ + 1:NH_ + 2])

            # CCE halves: nem -> masked1 -> Ln+accum
            for h in range(NH_):
                comb_t = comb_pool.tile([P, FREE_H], BF16, tag="comb")
                nc.sync.dma_start(comb_t[:], comb_d[h])

                idx_h = idxg[:, h * TH:(h + 1) * TH]
                idx4 = idx_h.rearrange("p (n o b) -> p n o b", o=1, b=BLK)
                iota4 = iota_cb[:].rearrange("p (o c b) -> p o c b",
                                             o=1, c=C, b=BLK)
                b_iota, b_idx = bass.broadcast_tensor_aps(iota4, idx4)

                nem = nem_pool.tile([P, FREE_H], BF16, tag="nem")
                nem_v = nem[:].rearrange("p (n c b) -> p n c b", c=C, b=BLK)
                nc.vector.tensor_tensor(nem_v, b_iota, b_idx, ne)

                msk = msk_pool.tile([P, FREE_H], BF16, tag="msk")
                nc.vector.tensor_tensor(msk[:], nem[:], comb_t[:], mx)

                # Ln output overwrites nem (dead after msk) to save SBUF
                nc.scalar.activation(nem[:], msk[:], Ln,
                                     accum_out=parts[:, h:h + 1])

            # total = -(cce0 + cce1) - W1*a1 - W0*a0
            rC = const_pool.tile([P, 1], F32)
            nc.vector.tensor_reduce(rC[:], parts[:, 0:NH_], axis=mybir.AxisListType.X,
                                    op=mybir.AluOpType.add)
            wA = const_pool.tile([P, 1], F32)
            nc.vector.tensor_scalar(wA[:], parts[:, NH_:NH_ + 1], -W1, None,
                                    mybir.AluOpType.mult)
            total = const_pool.tile([P, 1], F32)
            nc.vector.scalar_tensor_tensor(
                total[:], parts[:, NH_ + 1:NH_ + 2], -W0, wA[:],
                op0=mybir.AluOpType.mult, op1=mybir.AluOpType.add)
            nc.vector.tensor_tensor(total[:], total[:], rC[:],
                                    mybir.AluOpType.subtract)

            nc.sync.dma_start(out_d[:], total[:])

    nc.compile()
    return nc


_NC_CACHE = {}


def make_in_maps(y_pred_stroke, y_pred_comb, y_stroke, y_comb):
    y_pred_stroke = np.asarray(y_pred_stroke, dtype=np.float32)
    y_pred_comb = np.asarray(y_pred_comb, dtype=np.float32)
    y_stroke = np.asarray(y_stroke, dtype=np.float32)
    y_comb = np.asarray(y_comb)
    Bc = B // N_CORES
    in_maps = []
    for core in range(N_CORES):
        sl = slice(core * Bc, (core + 1) * Bc)
        # block-token layout: [P, T, C] -> [NH, P, nblk, C, BLK]
        cm = np.ascontiguousarray(y_pred_comb[sl]).reshape(P, T, C)
        cm = cm.reshape(P, NH, NBLK, BLK, C).transpose(1, 0, 2, 4, 3)
        comb = np.ascontiguousarray(cm).reshape(NH, P, NBLK * C * BLK)
        comb = comb.astype(ml_dtypes.bfloat16)

        idxg = (np.ascontiguousarray(y_comb[sl]).astype(np.float32)
                + (1.0 - np.ascontiguousarray(y_stroke[sl])[..., 0]) * BIG
                ).reshape(P, T)
        ps = np.ascontiguousarray(y_pred_stroke[sl]).reshape(P, T)
        sides = np.concatenate([idxg, ps], axis=1).astype(ml_dtypes.bfloat16)
        in_maps.append({"comb": comb, "sides": sides})
    return in_maps


def kernel(y_pred_stroke, y_pred_comb, y_stroke, y_comb):
    key = (NT, Tp)
    if key not in _NC_CACHE:
        _NC_CACHE[key] = _build(NH, T)
    nc = _NC_CACHE[key]
    in_maps = make_in_maps(y_pred_stroke, y_pred_comb, y_stroke, y_comb)
    res = run_bass_kernel_spmd(nc, in_maps, list(range(N_CORES)))
    total = 0.0
    for r in res.results:
        total += r["out"].astype(np.float64).sum()
    return np.asarray([total / (B * S)], dtype=np.float32)
